# revision 1
# baseline (speedup 1.0000x reference)
"""Trainium2 Bass kernel for single-head attention (B=8, S=2048, DIN=768, DOUT=64).

Strategy: pure data parallelism — one batch element per NeuronCore (8 cores).
Per core, attention is computed entirely in "transposed score" layout so that
no on-chip transposes of the [S,S] matrix are ever needed:

  qT     [64, S]   = Wq.T @ x.T              (from host-pretransposed xT)
  kT/v   per key tile, interleaved into the main loop (keeps TensorE dense)
  scT    [128, S]  = kT_tile.T @ qT          (k on partitions, q on free dim)
  e      = exp(scT * 1/sqrt(S)) * keepT      (keepT = ~mask.T, bf16, from host)
  ctxT   [65, S]  += v65_tile.T @ e          (v65 = [v | 1]; row 64 = softmax denom)
  out    [S, 64]   = (ctxT[:64] / ctxT[64]).T (16 small PE transposes at the end)

The softmax row-sum is obtained through the ones-column of v65 (a matmul
reduction over the partition axis), so no vector-engine reductions and no
row-max subtraction are needed (scores are tiny: |s| < ~0.5; masked lanes are
zeroed post-exp by the keep multiply).

The kT and v projections for key tile t are computed inside the main loop so
TensorE has ~2.4us of dense work per tile, matching ScalarE's exp pace — this
keeps the PE HAM clock-gate at 8/8 (2.4 GHz) instead of throttling to 1.2 GHz.
"""

import math
import sys
from contextlib import ExitStack

import numpy as np

sys.path.insert(0, "/opt/trn_rl_repo")

import ml_dtypes  # noqa: E402

import concourse.bass as bass  # noqa: E402
import concourse.tile as tile  # noqa: E402
from concourse import bacc, mybir  # noqa: E402
from concourse.bass import ds  # noqa: E402
from concourse.bass_utils import run_bass_kernel_spmd  # noqa: E402
from concourse.masks import make_identity  # noqa: E402

B, S, DIN, DOUT = 8, 2048, 768, 64
P = 128
NCH = DIN // P  # 6 contraction chunks for the projections
KT = S // P  # 16 key tiles
NS = 512  # matmul moving-operand free dim (one PSUM bank fp32)
H = 2  # exp halves per key tile
HQ = S // H  # 1024

F32 = mybir.dt.float32
BF16 = mybir.dt.bfloat16

_NC_CACHE = None


def build_nc():
    nc = bacc.Bacc("TRN2", target_bir_lowering=False, debug=False)

    xT = nc.declare_dram_parameter("xT", [DIN, S], BF16, isOutput=False)
    keep = nc.declare_dram_parameter("keep", [S, S], BF16, isOutput=False)
    wq = nc.declare_dram_parameter("wq", [DIN, DOUT], BF16, isOutput=False)
    wk = nc.declare_dram_parameter("wk", [DIN, DOUT], BF16, isOutput=False)
    wv = nc.declare_dram_parameter("wv", [DIN, DOUT], BF16, isOutput=False)
    bq = nc.declare_dram_parameter("bq", [DOUT, 1], F32, isOutput=False)
    bk = nc.declare_dram_parameter("bk", [DOUT, 1], F32, isOutput=False)
    out = nc.declare_dram_parameter("out", [S, DOUT], F32, isOutput=True)

    with tile.TileContext(nc) as tc, ExitStack() as ctx:
        singles = ctx.enter_context(tc.tile_pool(name="singles", bufs=1))
        epool = ctx.enter_context(tc.tile_pool(name="epool", bufs=4))
        opool = ctx.enter_context(tc.tile_pool(name="opool", bufs=4))

        # ---- constants / weights
        wq_sb = singles.tile([P, NCH, DOUT], BF16)
        nc.sync.dma_start(out=wq_sb, in_=wq.rearrange("(c p) m -> p c m", p=P))
        wk_sb = singles.tile([P, NCH, DOUT], BF16)
        nc.sync.dma_start(out=wk_sb, in_=wk.rearrange("(c p) m -> p c m", p=P))
        wv_sb = singles.tile([P, NCH, DOUT], BF16)
        nc.sync.dma_start(out=wv_sb, in_=wv.rearrange("(c p) m -> p c m", p=P))
        bq_sb = singles.tile([DOUT, 1], F32)
        nc.sync.dma_start(out=bq_sb, in_=bq[:, :])
        bk_sb = singles.tile([DOUT, 1], F32)
        nc.sync.dma_start(out=bk_sb, in_=bk[:, :])

        # ---- x.T resident (bf16): per-chunk DMAs so compute can start early
        xT_sb = singles.tile([P, NCH, S], BF16)
        for c in range(NCH):
            nc.sync.dma_start(out=xT_sb[:, c, :], in_=xT[ds(c * P, P), :])

        # ---- keep mask resident (bf16): [k(128 part), ktile, q]
        keep_sb = singles.tile([P, KT, S], BF16)
        for t in range(KT):
            nc.sync.dma_start(out=keep_sb[:, t, :], in_=keep[ds(t * P, P), :])

        ident = singles.tile([P, P], F32)
        make_identity(nc, ident)

        # ---- v with a ones column: [s(128 part), ktile, 65] bf16
        v65_sb = singles.tile([P, KT, DOUT + 1], BF16)
        nc.gpsimd.memset(v65_sb, 1.0)

        qT_sb = singles.tile([DOUT, S], BF16)
        kT_sb = singles.tile([DOUT, S], BF16)
        ctxT_sb = singles.tile([DOUT + 1, S], F32)
        inv_sqrt_s = float(1.0 / math.sqrt(S))

        with (
            tc.tile_pool(name="psA", bufs=2, space="PSUM") as psA,
            tc.tile_pool(name="psC", bufs=1, space="PSUM") as psC,
        ):
            # ---- qT projection, up front (dense PE work to warm the HAM)
            for h in range(H):
                q_ps = psA.tile([P, HQ], F32, tag="big")
                for n in range(HQ // NS):
                    for c in range(NCH):
                        nc.tensor.matmul(
                            q_ps[0:DOUT, ds(n * NS, NS)],
                            lhsT=wq_sb[:, c, :],
                            rhs=xT_sb[:, c, ds(h * HQ + n * NS, NS)],
                            start=(c == 0),
                            stop=(c == NCH - 1),
                        )
                nc.vector.tensor_scalar_add(
                    qT_sb[:, ds(h * HQ, HQ)], q_ps[0:DOUT, :], bq_sb
                )

            # ---- main loop over key tiles; kT/v projections interleaved
            ctx_ps = psC.tile([DOUT + 1, S], F32)
            for t in range(KT):
                # kT projection for this tile: [64, 128]
                kt_ps = psA.tile([P, HQ], F32, tag="big")
                for c in range(NCH):
                    nc.tensor.matmul(
                        kt_ps[0:DOUT, 0:P],
                        lhsT=wk_sb[:, c, :],
                        rhs=xT_sb[:, c, ds(t * P, P)],
                        start=(c == 0),
                        stop=(c == NCH - 1),
                    )
                nc.vector.tensor_scalar_add(
                    kT_sb[:, ds(t * P, P)], kt_ps[0:DOUT, 0:P], bk_sb
                )

                # v projection for this tile: [128, 64]
                v_ps = psA.tile([P, HQ], F32, tag="big")
                for c in range(NCH):
                    nc.tensor.matmul(
                        v_ps[:, 0:DOUT],
                        lhsT=xT_sb[:, c, ds(t * P, P)],
                        rhs=wv_sb[:, c, :],
                        start=(c == 0),
                        stop=(c == NCH - 1),
                    )
                nc.scalar.copy(v65_sb[:, t, 0:DOUT], v_ps[:, 0:DOUT])

                # scores + exp + mask + context, in two q halves
                for h in range(H):
                    sc = psA.tile([P, HQ], F32, tag="big")
                    for n in range(HQ // NS):
                        nc.tensor.matmul(
                            sc[:, ds(n * NS, NS)],
                            lhsT=kT_sb[:, ds(t * P, P)],
                            rhs=qT_sb[:, ds(h * HQ + n * NS, NS)],
                            start=True,
                            stop=True,
                        )
                    ex = epool.tile([P, HQ], BF16, tag="exp")
                    nc.scalar.activation(
                        out=ex,
                        in_=sc,
                        func=mybir.ActivationFunctionType.Exp,
                        scale=inv_sqrt_s,
                    )
                    nc.vector.tensor_mul(ex, ex, keep_sb[:, t, ds(h * HQ, HQ)])
                    for n in range(HQ // NS):
                        nc.tensor.matmul(
                            ctx_ps[:, ds(h * HQ + n * NS, NS)],
                            lhsT=v65_sb[:, t, :],
                            rhs=ex[:, ds(n * NS, NS)],
                            start=(t == 0),
                            stop=(t == KT - 1),
                        )

            # ---- epilogue: transpose ctxT back, normalize, store
            nc.vector.tensor_copy(ctxT_sb, ctx_ps)
            for t in range(KT):
                tr = psA.tile([P, DOUT + 1], F32, tag="big")
                nc.tensor.transpose(
                    tr,
                    ctxT_sb[:, ds(t * P, P)],
                    ident[0 : DOUT + 1, 0 : DOUT + 1],
                )
                rc = opool.tile([P, 1], F32, tag="rc")
                nc.vector.reciprocal(rc, tr[:, DOUT : DOUT + 1])
                g, gi = t // 4, t % 4
                if gi == 0:
                    ostage = opool.tile([P, 4, DOUT], F32, tag="ostage")
                nc.vector.tensor_scalar_mul(ostage[:, gi, :], tr[:, 0:DOUT], rc)
                if gi == 3:
                    nc.sync.dma_start(
                        out=out[ds(g * 4 * P, 4 * P), :].rearrange(
                            "(t p) m -> p t m", p=P
                        ),
                        in_=ostage,
                    )

    nc.finalize()
    return nc


def _get_nc():
    global _NC_CACHE
    if _NC_CACHE is None:
        _NC_CACHE = build_nc()
    return _NC_CACHE


def kernel(**inputs):
    x = np.asarray(inputs["input_tensor"], dtype=np.float32)  # [B, S, DIN]
    mask = np.asarray(inputs["attention_mask"])  # [B, S, S] bool
    Wq = np.asarray(inputs["Wq"], dtype=np.float32)
    Wk = np.asarray(inputs["Wk"], dtype=np.float32)
    Wv = np.asarray(inputs["Wv"], dtype=np.float32)
    bq = np.asarray(inputs["bq"], dtype=np.float32)
    bk = np.asarray(inputs["bk"], dtype=np.float32)
    bv = np.asarray(inputs["bv"], dtype=np.float32)

    wq_b = np.ascontiguousarray(Wq).astype(ml_dtypes.bfloat16)
    wk_b = np.ascontiguousarray(Wk).astype(ml_dtypes.bfloat16)
    wv_b = np.ascontiguousarray(Wv).astype(ml_dtypes.bfloat16)
    bq_c = np.ascontiguousarray(bq.reshape(DOUT, 1))
    bk_c = np.ascontiguousarray(bk.reshape(DOUT, 1))

    in_maps = []
    for b in range(B):
        xTb = np.ascontiguousarray(x[b].T).astype(ml_dtypes.bfloat16)  # [DIN, S]
        keepb = (~mask[b]).T.astype(ml_dtypes.bfloat16)  # [S, S], 1=keep
        in_maps.append(
            {
                "xT": xTb,
                "keep": np.ascontiguousarray(keepb),
                "wq": wq_b,
                "wk": wk_b,
                "wv": wv_b,
                "bq": bq_c,
                "bk": bk_c,
            }
        )

    nc = _get_nc()
    res = run_bass_kernel_spmd(nc, in_maps, core_ids=list(range(B)))
    out = np.stack([np.asarray(res.results[b]["out"], np.float32) for b in range(B)])
    out = out + bv[None, None, :]
    return out.astype(np.float32)

